# revision 6
# baseline (speedup 1.0000x reference)
"""Trainium2 Bass kernel for y = 2*(einsum('bct,oc->bot', pre, W_pre) + b_pre).

Shapes (hardcoded): pre [16, 512, 4096] f32, W_pre [512, 512] f32, b_pre [512] f32.
Sharding: data-parallel over B across 8 cores (2 batches per core).

Per core: out[b, o, t] = 2*(sum_c W[o,c]*pre[b,c,t] + bias[o]) for 2 batches.
PE matmul computes lhsT.T @ rhs with lhsT = W.T tiles [K=128, M=128] and
rhs = pre tiles [K=128, N=512]; accumulate 4 K-tiles into one PSUM bank,
then ScalarE/VectorE apply out = psum + 2*bias on eviction PSUM->SBUF.

All HBM traffic is bf16 (host casts in/out) and all DRAM buffers are
host-packed so each DMA descriptor row is 4KB contiguous:
 - x: [BPC, NCH, 128, KT*512]  (chunk ci holds all 4 K-tiles of 512 t-cols)
 - w: [128, MT*KT*128]          (single 512KB load, all 16 weight tiles)
 - out: [BPC, 2, MT, 128, 2048] (group-major; batch-1 groups stored in
   tapered sub-blocks so the final DMAs are small)
A few fp32 warm-up matmuls on a memset tile run at t=0 so the PE HAM
clock-gate reaches 2.4GHz before the real matmul stream starts.
"""

import os
import sys

for _p in ("/opt/trn_rl_repo", "/root/.axon_site/_ro/trn_rl_repo"):
    if os.path.isdir(_p) and _p not in sys.path:
        sys.path.append(_p)

from contextlib import ExitStack

import numpy as np
import ml_dtypes

import concourse.bass as bass
import concourse.tile as tile
from concourse import bacc, mybir
from concourse.bass_utils import run_bass_kernel_spmd

B, C, T = 16, 512, 4096  # batch, channels (in == out), sequence
NCORES = 8
BPC = B // NCORES  # batches per core
P = 128
KT = C // P  # contraction tiles
MT = C // P  # output-channel tiles
NCHUNK = 512  # matmul moving-operand free dim (PSUM bank = 512 fp32)
NCH = T // NCHUNK  # input chunks per batch
XW = KT * NCHUNK  # packed x-chunk width (all K-tiles side by side)
GCOLS = 2048  # output group block width
NG = T // GCOLS  # output groups per batch
# Output store sub-blocks (cols) per batch: taper the last batch so the
# final DMAs after the last matmul are small.
OSUB = {0: [[2048], [2048]], 1: [[2048], [1024, 512, 512]]}

IN_DT = mybir.dt.bfloat16
OUT_DT = mybir.dt.bfloat16
WARMUP_MMS = 2  # fp32 N=512 matmuls (~1.7us each cold) to trip the HAM gate

LAST_RESULT = None  # BassKernelResults of the most recent run (for test harness)
_cache = {}


def _build():
    # Bacc (not plain Bass): its finalize() runs move_matmul_waits_to_ldweights +
    # generate_event_semaphores, which walrus needs.
    nc = bacc.Bacc("TRN2", target_bir_lowering=False, debug=False, num_devices=NCORES)
    xp = nc.dram_tensor("xp", [BPC, NCH, P, XW], IN_DT, kind="ExternalInput").ap()
    wt = nc.dram_tensor("wt", [P, MT * KT * P], IN_DT, kind="ExternalInput").ap()
    b2 = nc.dram_tensor("b2", [P, MT], mybir.dt.float32, kind="ExternalInput").ap()
    out = nc.dram_tensor(
        "out", [BPC, NG, MT, P, GCOLS], OUT_DT, kind="ExternalOutput"
    ).ap()

    with ExitStack() as ctx:
        tc = ctx.enter_context(tile.TileContext(nc))
        wpool = ctx.enter_context(tc.tile_pool(name="w", bufs=1))
        bpool = ctx.enter_context(tc.tile_pool(name="bias", bufs=1))
        dpool = ctx.enter_context(tc.tile_pool(name="dummy", bufs=1))
        xpool = ctx.enter_context(tc.tile_pool(name="x", bufs=2))
        opool = ctx.enter_context(tc.tile_pool(name="o", bufs=8))
        pspool = ctx.enter_context(tc.tile_pool(name="ps", bufs=8, space="PSUM"))

        # PE warm-up: memset a scratch fp32 tile, then a few self-contained
        # matmuls on it.  They depend only on the memset, so they run during
        # the initial DMA fill and flip the HAM clock-gate to 8/8 before the
        # first real matmul issues.
        if WARMUP_MMS:
            dummy = dpool.tile([P, P + NCHUNK], mybir.dt.float32)
            nc.vector.memset(dummy[:], 0.0)
            for i in range(WARMUP_MMS):
                psw = pspool.tile([P, NCHUNK], mybir.dt.float32, tag="ps")
                nc.tensor.matmul(
                    psw[:],
                    dummy[:, 0:P],
                    dummy[:, P : P + NCHUNK],
                    start=True,
                    stop=True,
                )

        # W.T in 4 per-mt tiles so the first matmul group only waits for
        # mt=0's 128KB (+ x chunk 0), not the full 512KB.
        wmt = [wpool.tile([P, KT * P], IN_DT, name=f"w_{mt}") for mt in range(MT)]
        wtiles = [
            [wmt[mt][:, kt * P : (kt + 1) * P] for mt in range(MT)]
            for kt in range(KT)
        ]
        nc.sync.dma_start(wmt[0][:], wt[0:P, 0 : KT * P])

        # Batch 0 x chunks up front (consumption order); batch 1 inside loop.
        xtiles = {}
        x = xpool.tile([P, XW], IN_DT, name="x_0_0", tag="x0")
        nc.sync.dma_start(x[:], xp[0, 0, 0:P, 0:XW])
        xtiles[(0, 0)] = x

        for mt in range(1, MT):
            nc.sync.dma_start(wmt[mt][:], wt[0:P, mt * KT * P : (mt + 1) * KT * P])

        btile = bpool.tile([P, MT], mybir.dt.float32)
        nc.sync.dma_start(btile[:], b2[:])

        for ci in range(1, NCH):
            x = xpool.tile([P, XW], IN_DT, name=f"x_0_{ci}", tag=f"x{ci}")
            nc.sync.dma_start(x[:], xp[0, ci, 0:P, 0:XW])
            xtiles[(0, ci)] = x

        for b in range(BPC):
            if b > 0:
                for ci in range(NCH):
                    x = xpool.tile([P, XW], IN_DT, name=f"x_{b}_{ci}", tag=f"x{ci}")
                    nc.sync.dma_start(x[:], xp[b, ci, 0:P, 0:XW])
                    xtiles[(b, ci)] = x

            for g in range(NG):
                subs = OSUB[b][g]
                last_group = b == BPC - 1 and g == NG - 1
                c0 = 0
                for s, cols in enumerate(subs):
                    # Per-sub-block output tiles: the store only depends on
                    # this sub-block's evictions, so it issues immediately.
                    otiles = [
                        opool.tile(
                            [P, cols], OUT_DT,
                            name=f"o_{b}_{g}_{s}_{mt}", tag=f"o{s}",
                            bufs=(8 if s == 0 else 2),
                        )
                        for mt in range(MT)
                    ]
                    for jj in range(cols // NCHUNK):
                        ci = (g * GCOLS + c0) // NCHUNK + jj
                        xt = xtiles[(b, ci)]
                        for mt in range(MT):
                            ps = pspool.tile([P, NCHUNK], mybir.dt.float32, tag="ps")
                            for kt in range(KT):
                                nc.tensor.matmul(
                                    ps[:],
                                    wtiles[kt][mt],
                                    xt[:, kt * NCHUNK : (kt + 1) * NCHUNK],
                                    start=(kt == 0),
                                    stop=(kt == KT - 1),
                                )
                            # W is pre-scaled by 2 on the host, so only + 2*bias
                            # remains; alternate DVE/ACT so neither engine binds.
                            dst = otiles[mt][:, bass.ts(jj, NCHUNK)]
                            bias_col = btile[:, mt : mt + 1]
                            if mt % 2 == 0:
                                nc.vector.tensor_scalar_add(dst, ps[:], bias_col)
                            else:
                                nc.scalar.activation(
                                    dst,
                                    ps[:],
                                    mybir.ActivationFunctionType.Identity,
                                    bias=bias_col,
                                )
                    for mt in range(MT):
                        # Final group: split store issue between the SWDGE
                        # (gpsimd) and HWDGE (sync) paths so the last few
                        # stores don't serialize on one sequencer.
                        eng = nc.sync if last_group and mt % 2 == 0 else nc.gpsimd
                        eng.dma_start(
                            out[b, g, mt, 0:P, bass.ds(c0, cols)],
                            otiles[mt][:],
                        )
                    c0 += cols
    # The axon/PJRT exec path serializes nc as-is; finalize here so Bacc's
    # compile passes (register alloc, event-semaphore wait splitting) run.
    nc.finalize()
    return nc


def kernel(pre, W_pre, b_pre):
    global LAST_RESULT
    bf16 = ml_dtypes.bfloat16
    pre16 = np.asarray(pre, dtype=np.float32).astype(bf16)
    # xp[b, ci, p, kt*512 + j] = pre[b, kt*128+p, ci*512+j]
    xp = np.ascontiguousarray(
        pre16.reshape(B, KT, P, NCH, NCHUNK).transpose(0, 3, 2, 1, 4)
    ).reshape(B, NCH, P, XW)
    # Fold the reference's final y+y into the weights/bias: out = (2W)x + 2b.
    # wt[p, mt*512 + kt*128 + j] = 2*W[mt*128+j, kt*128+p]
    w4 = (2.0 * np.asarray(W_pre, dtype=np.float32)).astype(bf16)
    wt = np.ascontiguousarray(
        w4.reshape(MT, P, KT, P).transpose(3, 0, 2, 1)
    ).reshape(P, MT * KT * P)
    b2 = np.ascontiguousarray(
        (2.0 * np.asarray(b_pre, dtype=np.float32)).reshape(MT, P).T
    )
    if "nc" not in _cache:
        _cache["nc"] = _build()
    nc = _cache["nc"]
    in_maps = [
        {"xp": xp[i * BPC : (i + 1) * BPC], "wt": wt, "b2": b2}
        for i in range(NCORES)
    ]
    res = run_bass_kernel_spmd(nc, in_maps, list(range(NCORES)))
    LAST_RESULT = res
    # out[b, g, mt, p, c] -> y[b, mt*128+p, g*2048+c]
    o = np.concatenate([res.results[i]["out"] for i in range(NCORES)], axis=0)
    return np.ascontiguousarray(
        o.transpose(0, 2, 3, 1, 4).reshape(B, C, T).astype(np.float32)
    )


# revision 7
# speedup vs baseline: 1.0152x; 1.0152x over previous
"""Trainium2 Bass kernel for y = 2*(einsum('bct,oc->bot', pre, W_pre) + b_pre).

Shapes (hardcoded): pre [16, 512, 4096] f32, W_pre [512, 512] f32, b_pre [512] f32.
Sharding: data-parallel over B across 8 cores (2 batches per core).

Per core: out[b, o, t] = 2*(sum_c W[o,c]*pre[b,c,t] + bias[o]) for 2 batches.
PE matmul computes lhsT.T @ rhs with lhsT = W.T tiles [K=128, M=128] and
rhs = pre tiles [K=128, N=512]; accumulate 4 K-tiles into one PSUM bank,
then ScalarE/VectorE apply out = psum + 2*bias on eviction PSUM->SBUF.

All HBM traffic is bf16 (host casts in/out) and all DRAM buffers are
host-packed so each DMA descriptor row is 4KB contiguous:
 - x: [BPC, NCH, 128, KT*512]  (chunk ci holds all 4 K-tiles of 512 t-cols)
 - w: [128, MT*KT*128]          (single 512KB load, all 16 weight tiles)
 - out: [BPC, 2, MT, 128, 2048] (group-major; batch-1 groups stored in
   tapered sub-blocks so the final DMAs are small)
A few fp32 warm-up matmuls on a memset tile run at t=0 so the PE HAM
clock-gate reaches 2.4GHz before the real matmul stream starts.
"""

import os
import sys

for _p in ("/opt/trn_rl_repo", "/root/.axon_site/_ro/trn_rl_repo"):
    if os.path.isdir(_p) and _p not in sys.path:
        sys.path.append(_p)

from contextlib import ExitStack

import numpy as np
import ml_dtypes

import concourse.bass as bass
import concourse.tile as tile
from concourse import bacc, mybir
from concourse.bass_utils import run_bass_kernel_spmd

B, C, T = 16, 512, 4096  # batch, channels (in == out), sequence
NCORES = 8
BPC = B // NCORES  # batches per core
P = 128
KT = C // P  # contraction tiles
MT = C // P  # output-channel tiles
NCHUNK = 512  # matmul moving-operand free dim (PSUM bank = 512 fp32)
NCH = T // NCHUNK  # input chunks per batch
XW = KT * NCHUNK  # packed x-chunk width (all K-tiles side by side)
GCOLS = 2048  # output group block width
NG = T // GCOLS  # output groups per batch
# Output store sub-blocks (cols) per batch: taper the last batch so the
# final DMAs after the last matmul are small.
OSUB = {0: [[2048], [2048]], 1: [[2048], [1024, 512, 512]]}

IN_DT = mybir.dt.bfloat16
OUT_DT = mybir.dt.bfloat16
WARMUP_MMS = 2  # fp32 N=512 matmuls (~1.7us each cold) to trip the HAM gate

LAST_RESULT = None  # BassKernelResults of the most recent run (for test harness)
_cache = {}


def _build():
    # Bacc (not plain Bass): its finalize() runs move_matmul_waits_to_ldweights +
    # generate_event_semaphores, which walrus needs.
    nc = bacc.Bacc("TRN2", target_bir_lowering=False, debug=False, num_devices=NCORES)
    xp = nc.dram_tensor("xp", [BPC, NCH, P, XW], IN_DT, kind="ExternalInput").ap()
    wt = nc.dram_tensor("wt", [P, MT * KT * P], IN_DT, kind="ExternalInput").ap()
    b2 = nc.dram_tensor("b2", [P, MT], mybir.dt.float32, kind="ExternalInput").ap()
    out = nc.dram_tensor(
        "out", [BPC, NG, MT, P, GCOLS], OUT_DT, kind="ExternalOutput"
    ).ap()

    with ExitStack() as ctx:
        tc = ctx.enter_context(tile.TileContext(nc))
        wpool = ctx.enter_context(tc.tile_pool(name="w", bufs=1))
        bpool = ctx.enter_context(tc.tile_pool(name="bias", bufs=1))
        dpool = ctx.enter_context(tc.tile_pool(name="dummy", bufs=1))
        xpool = ctx.enter_context(tc.tile_pool(name="x", bufs=2))
        opool = ctx.enter_context(tc.tile_pool(name="o", bufs=8))
        pspool = ctx.enter_context(tc.tile_pool(name="ps", bufs=8, space="PSUM"))

        # PE warm-up: memset a scratch fp32 tile, then a few self-contained
        # matmuls on it.  They depend only on the memset, so they run during
        # the initial DMA fill and flip the HAM clock-gate to 8/8 before the
        # first real matmul issues.
        if WARMUP_MMS:
            dummy = dpool.tile([P, P + NCHUNK], mybir.dt.float32)
            nc.vector.memset(dummy[:], 0.0)
            for i in range(WARMUP_MMS):
                psw = pspool.tile([P, NCHUNK], mybir.dt.float32, tag="ps")
                nc.tensor.matmul(
                    psw[:],
                    dummy[:, 0:P],
                    dummy[:, P : P + NCHUNK],
                    start=True,
                    stop=True,
                )

        # W.T in 4 per-mt tiles so the first matmul group only waits for
        # mt=0's 128KB (+ x chunk 0), not the full 512KB.
        wmt = [wpool.tile([P, KT * P], IN_DT, name=f"w_{mt}") for mt in range(MT)]
        wtiles = [
            [wmt[mt][:, kt * P : (kt + 1) * P] for mt in range(MT)]
            for kt in range(KT)
        ]
        nc.sync.dma_start(wmt[0][:], wt[0:P, 0 : KT * P])

        # Batch 0 x chunks up front (consumption order); batch 1 inside loop.
        xtiles = {}
        x = xpool.tile([P, XW], IN_DT, name="x_0_0", tag="x0")
        nc.sync.dma_start(x[:], xp[0, 0, 0:P, 0:XW])
        xtiles[(0, 0)] = x

        for mt in range(1, MT):
            nc.sync.dma_start(wmt[mt][:], wt[0:P, mt * KT * P : (mt + 1) * KT * P])

        btile = bpool.tile([P, MT], mybir.dt.float32)
        nc.sync.dma_start(btile[:], b2[:])

        for ci in range(1, NCH):
            x = xpool.tile([P, XW], IN_DT, name=f"x_0_{ci}", tag=f"x{ci}")
            nc.sync.dma_start(x[:], xp[0, ci, 0:P, 0:XW])
            xtiles[(0, ci)] = x

        for b in range(BPC):
            if b > 0:
                for ci in range(NCH):
                    x = xpool.tile([P, XW], IN_DT, name=f"x_{b}_{ci}", tag=f"x{ci}")
                    nc.sync.dma_start(x[:], xp[b, ci, 0:P, 0:XW])
                    xtiles[(b, ci)] = x

            for g in range(NG):
                otiles = [
                    opool.tile([P, GCOLS], OUT_DT, name=f"o_{b}_{g}_{mt}", tag="o")
                    for mt in range(MT)
                ]
                for j in range(GCOLS // NCHUNK):
                    ci = g * (GCOLS // NCHUNK) + j
                    xt = xtiles[(b, ci)]
                    for mt in range(MT):
                        ps = pspool.tile([P, NCHUNK], mybir.dt.float32, tag="ps")
                        for kt in range(KT):
                            nc.tensor.matmul(
                                ps[:],
                                wtiles[kt][mt],
                                xt[:, kt * NCHUNK : (kt + 1) * NCHUNK],
                                start=(kt == 0),
                                stop=(kt == KT - 1),
                            )
                        # W is pre-scaled by 2 on the host, so only + 2*bias
                        # remains; alternate DVE/ACT so neither engine binds.
                        dst = otiles[mt][:, bass.ts(j, NCHUNK)]
                        bias_col = btile[:, mt : mt + 1]
                        if mt % 2 == 0:
                            nc.vector.tensor_scalar_add(dst, ps[:], bias_col)
                        else:
                            nc.scalar.activation(
                                dst,
                                ps[:],
                                mybir.ActivationFunctionType.Identity,
                                bias=bias_col,
                            )
                # Store the group; batch-1's last group in tapered sub-blocks.
                for mt in range(MT):
                    c0 = 0
                    for cols in OSUB[b][g]:
                        nc.gpsimd.dma_start(
                            out[b, g, mt, 0:P, bass.ds(c0, cols)],
                            otiles[mt][:, bass.ds(c0, cols)],
                        )
                        c0 += cols
    # The axon/PJRT exec path serializes nc as-is; finalize here so Bacc's
    # compile passes (register alloc, event-semaphore wait splitting) run.
    nc.finalize()
    return nc


def kernel(pre, W_pre, b_pre):
    global LAST_RESULT
    bf16 = ml_dtypes.bfloat16
    pre16 = np.asarray(pre, dtype=np.float32).astype(bf16)
    # xp[b, ci, p, kt*512 + j] = pre[b, kt*128+p, ci*512+j]
    xp = np.ascontiguousarray(
        pre16.reshape(B, KT, P, NCH, NCHUNK).transpose(0, 3, 2, 1, 4)
    ).reshape(B, NCH, P, XW)
    # Fold the reference's final y+y into the weights/bias: out = (2W)x + 2b.
    # wt[p, mt*512 + kt*128 + j] = 2*W[mt*128+j, kt*128+p]
    w4 = (2.0 * np.asarray(W_pre, dtype=np.float32)).astype(bf16)
    wt = np.ascontiguousarray(
        w4.reshape(MT, P, KT, P).transpose(3, 0, 2, 1)
    ).reshape(P, MT * KT * P)
    b2 = np.ascontiguousarray(
        (2.0 * np.asarray(b_pre, dtype=np.float32)).reshape(MT, P).T
    )
    if "nc" not in _cache:
        _cache["nc"] = _build()
    nc = _cache["nc"]
    in_maps = [
        {"xp": xp[i * BPC : (i + 1) * BPC], "wt": wt, "b2": b2}
        for i in range(NCORES)
    ]
    res = run_bass_kernel_spmd(nc, in_maps, list(range(NCORES)))
    LAST_RESULT = res
    # out[b, g, mt, p, c] -> y[b, mt*128+p, g*2048+c]
    o = np.concatenate([res.results[i]["out"] for i in range(NCORES)], axis=0)
    return np.ascontiguousarray(
        o.transpose(0, 2, 3, 1, 4).reshape(B, C, T).astype(np.float32)
    )
